# revision 1
# baseline (speedup 1.0000x reference)
"""Trainium2 Bass kernel for nn_ContMixT (dense_cnn).

Data-parallel over batch: 8 samples -> 8 NeuronCores, no collectives.

Per-core pipeline (sample b):
  conv1: 3x3 dil=2 pad=2, 768->256, relu   (bf16 matmuls, fp32 PSUM accum)
  conv2: 3x3 dil=4 pad=4, 256->256, relu   (bf16), fused global-avg-pool
  FC chain: g_conv 1x1 + fc1 + fc2 + silu  -> per-channel 3x3 kernels wk
  dynamic depthwise 3x3 via diag(wk) matmuls (bf16)
  alpha = 0.3+0.4*sigmoid(1x1 conv over [f_mod, f_prev])
  out = alpha*f_mod + (1-alpha)*f_prev     (fp32, f_prev exact)

Spatial layout: conv activations live as padded 64x64 frames per channel
(SBUF tiles [128, 64, 64], zero ring of 4); convs run on interior-only
chunks of 7 rows x 56 cols (N=392) as 9 shifted matmuls per cin-block, so
jax-style zero padding falls out for free.  Conv biases are folded into the
PSUM accumulation as rank-1 matmuls (bias_row^T x ones_row) to keep
per-instruction semaphore waits within ISA limits.  All SBUF pools stay
open for the whole kernel (no address reuse -> no freed-zone WAR fan-in).
"""

import sys

if "/opt/trn_rl_repo" not in sys.path:
    sys.path.insert(0, "/opt/trn_rl_repo")

import numpy as np
import ml_dtypes

import concourse.bass as bass
import concourse.bacc as bacc
import concourse.tile as tile
from concourse import mybir
from concourse.bass_utils import run_bass_kernel_spmd

BF16 = ml_dtypes.bfloat16

B, C, H, W = 8, 256, 56, 56
HID = 256
P = 128
HP = 64          # padded frame side (pad ring of 4)
NCHUNK = 8       # 8 chunks x 7 rows
CROWS = 7
NFREE = CROWS * W  # 392

LAST_INFO = {}


def _taps(d):
    return [(ky * 3 + kx, (ky - 1) * d, (kx - 1) * d) for ky in range(3) for kx in range(3)]


def build_nc(repeat=1):
    nc = bacc.Bacc()
    f32 = mybir.dt.float32
    bf16 = mybir.dt.bfloat16

    # ---- dram I/O ----
    xprev = nc.dram_tensor("xprev", [4, P, HP, HP], bf16, kind="ExternalInput")   # padded f_tm2 b0,b1; f_tm1 b0,b1
    xt = nc.dram_tensor("xt", [2, P, HP, HP], bf16, kind="ExternalInput")         # padded f_t
    x1r = nc.dram_tensor("x1r", [2, P, H, W], f32, kind="ExternalInput")          # f_tm1 raw fp32
    x2r = nc.dram_tensor("x2r", [2, P, H, W], f32, kind="ExternalInput")          # f_tm2 raw fp32
    w1t = nc.dram_tensor("w1t", [6, P, 9 * HID], bf16, kind="ExternalInput")
    w2t = nc.dram_tensor("w2t", [2, P, 9 * HID], bf16, kind="ExternalInput")
    gwt = nc.dram_tensor("gwt", [2, P, C], f32, kind="ExternalInput")             # gw.T/3136
    fc1wt = nc.dram_tensor("fc1wt", [4, P, 512], f32, kind="ExternalInput")       # fc1_w.T (local half /3136)
    fc2wt = nc.dram_tensor("fc2wt", [4, P, C * 9], bf16, kind="ExternalInput")    # fc2_w.T
    b1row = nc.dram_tensor("b1row", [2, 1, P], bf16, kind="ExternalInput")
    b2row = nc.dram_tensor("b2row", [2, 1, P], bf16, kind="ExternalInput")
    gbrow = nc.dram_tensor("gbrow", [2, 1, P], f32, kind="ExternalInput")
    fc1bc = nc.dram_tensor("fc1bc", [P, 4], f32, kind="ExternalInput")
    fc2br = nc.dram_tensor("fc2br", [1, C * 9], bf16, kind="ExternalInput")
    awmc = nc.dram_tensor("awmc", [2, P, 1], bf16, kind="ExternalInput")          # aw[:256] (bf16, f_mod half)
    awpc = nc.dram_tensor("awpc", [2, P, 1], f32, kind="ExternalInput")           # 0.5*aw[256:] (f32, s half)
    abc = nc.dram_tensor("abc", [1, 1], f32, kind="ExternalInput")
    identc = nc.dram_tensor("identc", [P, P], bf16, kind="ExternalInput")
    onesc = nc.dram_tensor("onesc", [1, P], f32, kind="ExternalInput")
    o392b = nc.dram_tensor("o392b", [1, NFREE], bf16, kind="ExternalInput")
    o392f = nc.dram_tensor("o392f", [1, NFREE], f32, kind="ExternalInput")

    y = nc.dram_tensor("y", [2, P, H * W], f32, kind="ExternalOutput")
    wkd = nc.dram_tensor("wkd", [2, P, 9], f32)  # transpose bounce
    import os
    DBG = bool(os.environ.get("BASSDBG"))
    if DBG:
        dbg_y1 = nc.dram_tensor("dbg_y1", [2, P, HP * HP], bf16, kind="ExternalOutput")
        dbg_fcin = nc.dram_tensor("dbg_fcin", [P, 4], f32, kind="ExternalOutput")
        dbg_wks = nc.dram_tensor("dbg_wks", [P, 18], f32, kind="ExternalOutput")
        dbg_fm = nc.dram_tensor("dbg_fm", [2, P, H * W], bf16, kind="ExternalOutput")
        dbg_s = nc.dram_tensor("dbg_s", [2, P, H * W], f32, kind="ExternalOutput")

    Relu = mybir.ActivationFunctionType.Relu
    Sigmoid = mybir.ActivationFunctionType.Sigmoid
    Silu = mybir.ActivationFunctionType.Silu
    mult = mybir.AluOpType.mult
    add = mybir.AluOpType.add

    def r0(c):
        return 4 + CROWS * c

    with tile.TileContext(nc) as tc:
        with (
            tc.tile_pool(name="mp", bufs=1) as mp,
            tc.tile_pool(name="psb", bufs=4, space="PSUM") as psb,
            tc.tile_pool(name="pss", bufs=2, space="PSUM") as pss,
            tc.tile_pool(name="psr", bufs=2, space="PSUM") as psr,
        ):
            # ---------- tiles ----------
            xf = [mp.tile([P, HP, HP], bf16, name=f"xf{j}") for j in range(2)]
            xc = [mp.tile([P, HP, HP], bf16, name=f"xc{j}") for j in range(4)]
            y1 = [mp.tile([P, HP, HP], bf16, name=f"y1_{j}") for j in range(2)]
            w1s = [mp.tile([P, 9 * HID], bf16, name=f"w1s{j}") for j in range(6)]
            w2s = [mp.tile([P, 9 * HID], bf16, name=f"w2s{j}") for j in range(2)]
            gws = [mp.tile([P, C], f32, name=f"gws{j}") for j in range(2)]
            fc1ws = [mp.tile([P, 512], f32, name=f"fc1ws{j}") for j in range(4)]
            fc2ws = [mp.tile([P, C * 9], bf16, name=f"fc2ws{j}") for j in range(4)]
            fc2bs = mp.tile([1, C * 9], bf16, name="fc2bs")
            wrow = mp.tile([1, C * 9], f32, name="wrow")
            b1rs = [mp.tile([1, P], bf16, name=f"b1rs{j}") for j in range(2)]
            b2rs = [mp.tile([1, P], bf16, name=f"b2rs{j}") for j in range(2)]
            gbrs = [mp.tile([1, P], f32, name=f"gbrs{j}") for j in range(2)]
            awms = [mp.tile([P, 1], bf16, name=f"awms{j}") for j in range(2)]
            awps = [mp.tile([P, 1], f32, name=f"awps{j}") for j in range(2)]
            abs_ = mp.tile([1, 1], f32, name="abs_")
            ident = mp.tile([P, P], bf16, name="ident")
            ones = mp.tile([1, P], f32, name="ones")
            ones392b = mp.tile([1, NFREE], bf16, name="ones392b")
            ones392f = mp.tile([1, NFREE], f32, name="ones392f")
            diag = mp.tile([P, 18, P], bf16, name="diag")
            fc1bs = mp.tile([P, 4], f32, name="fc1bs")
            pacc = [mp.tile([P, NCHUNK], f32, name=f"pacc{j}") for j in range(2)]
            gsum = mp.tile([P, 2], f32, name="gsum")
            fcin = mp.tile([P, 4], f32, name="fcin")
            hsb = mp.tile([P, 4], f32, name="hsb")
            hb16 = mp.tile([P, 4], bf16, name="hb16")
            wks = mp.tile([P, 18], f32, name="wks")
            s = [mp.tile([P, H, W], f32, name=f"s{j}") for j in range(2)]
            fm = [mp.tile([P, H, W], bf16, name=f"fm{j}") for j in range(2)]

            for _rep in range(repeat):
                # ---------- loads ----------
                for j in range(2):
                    nc.sync.dma_start(out=b1rs[j], in_=b1row[j])
                    nc.sync.dma_start(out=b2rs[j], in_=b2row[j])
                    nc.sync.dma_start(out=gbrs[j], in_=gbrow[j])
                    nc.sync.dma_start(out=awms[j], in_=awmc[j])
                    nc.sync.dma_start(out=awps[j], in_=awpc[j])
                nc.sync.dma_start(out=ident, in_=identc[:, :])
                nc.sync.dma_start(out=ones, in_=onesc[:, :])
                nc.sync.dma_start(out=abs_, in_=abc[:, :])
                nc.sync.dma_start(out=fc1bs, in_=fc1bc[:, :])
                nc.sync.dma_start(out=ones392b, in_=o392b[:, :])
                nc.sync.dma_start(out=ones392f, in_=o392f[:, :])
                nc.sync.dma_start(out=fc2bs, in_=fc2br[:, :])
                for j in range(2):
                    nc.sync.dma_start(out=gws[j], in_=gwt[j])
                    nc.sync.dma_start(out=w2s[j], in_=w2t[j])
                for j in range(4):
                    nc.sync.dma_start(out=fc1ws[j], in_=fc1wt[j])
                    nc.sync.dma_start(out=fc2ws[j], in_=fc2wt[j])
                for j in range(2):
                    nc.sync.dma_start(out=xf[j], in_=xt[j])
                for j in range(6):
                    nc.sync.dma_start(out=w1s[j], in_=w1t[j])
                for j in range(4):
                    nc.sync.dma_start(out=xc[j], in_=xprev[j])
                for j in range(2):
                    nc.scalar.memzero(y1[j])
                # f_prev source: s = x1 + x2 (fp32, exact)
                for j in range(2):
                    nc.sync.dma_start(out=s[j], in_=x1r[j])
                    for q in range(4):
                        x2tq = mp.tile([P, 14, W], f32, name=f"x2t{j}{q}", tag="x2t", bufs=2)
                        nc.sync.dma_start(out=x2tq, in_=x2r[j][:, 14 * q: 14 * q + 14, :])
                        nc.vector.tensor_add(
                            s[j][:, 14 * q: 14 * q + 14, :],
                            s[j][:, 14 * q: 14 * q + 14, :],
                            x2tq,
                        )

                if DBG:
                    for j in range(2):
                        nc.sync.dma_start(out=dbg_s[j], in_=s[j].rearrange("p a b -> p (a b)"))

                # ---------- conv1 ----------
                cin_tiles = [xc[0], xc[1], xc[2], xc[3], xf[0], xf[1]]
                taps1 = _taps(2)
                for o in range(2):
                    for c in range(NCHUNK):
                        ps = psb.tile([P, NFREE], f32, name=f"psc1_{o}_{c}", tag="psb")
                        for ci in range(6):
                            xv = cin_tiles[ci]
                            for (t, dy, dx) in taps1:
                                nc.tensor.matmul(
                                    ps,
                                    w1s[ci][:, t * HID + o * P: t * HID + o * P + P],
                                    xv[:, r0(c) + dy: r0(c) + dy + CROWS, 4 + dx: 60 + dx],
                                    start=(ci == 0 and t == 0), stop=False,
                                )
                        nc.tensor.matmul(ps, b1rs[o], ones392b, start=False, stop=True)
                        nc.scalar.activation(
                            out=y1[o][:, r0(c): r0(c) + CROWS, 4:60],
                            in_=ps, func=Relu,
                        )

                # ---------- conv2 + pooled accumulation ----------
                taps2 = _taps(4)
                for o in range(2):
                    for c in range(NCHUNK):
                        ps = psb.tile([P, NFREE], f32, name=f"psc2_{o}_{c}", tag="psb")
                        for ci in range(2):
                            for (t, dy, dx) in taps2:
                                nc.tensor.matmul(
                                    ps,
                                    w2s[ci][:, t * HID + o * P: t * HID + o * P + P],
                                    y1[ci][:, r0(c) + dy: r0(c) + dy + CROWS, 4 + dx: 60 + dx],
                                    start=(ci == 0 and t == 0), stop=False,
                                )
                        nc.tensor.matmul(ps, b2rs[o], ones392b, start=False, stop=True)
                        sc2 = mp.tile([P, NFREE], bf16, name=f"sc2_{o}_{c}", tag="sc2", bufs=2)
                        nc.scalar.activation(
                            out=sc2, in_=ps, func=Relu,
                            accum_out=pacc[o][:, c: c + 1],
                        )

                # ---------- global pools ----------
                for o in range(2):
                    nc.vector.tensor_reduce(
                        out=gsum[:, o: o + 1], in_=pacc[o],
                        axis=mybir.AxisListType.X, op=add,
                    )
                for j in range(2):
                    nc.vector.tensor_reduce(
                        out=fcin[:, 2 + j: 3 + j], in_=xf[j][:, 4:60, 4:60],
                        axis=mybir.AxisListType.XY, op=add,
                    )

                # ---------- g_conv 1x1 ----------
                psg = pss.tile([P, 2], f32, name="psg", tag="pss")
                for m in range(2):
                    for k in range(2):
                        nc.tensor.matmul(
                            psg[:, m: m + 1],
                            gws[k][:, m * P: (m + 1) * P],
                            gsum[:, k: k + 1],
                            start=(k == 0), stop=False,
                        )
                    nc.tensor.matmul(psg[:, m: m + 1], gbrs[m], ones[:, 0:1],
                                     start=False, stop=True)
                    nc.vector.tensor_copy(fcin[:, m: m + 1], psg[:, m: m + 1])

                # ---------- fc1 ----------
                psh = pss.tile([P, 4], f32, name="psh", tag="pss")
                for m in range(4):
                    for k in range(4):
                        nc.tensor.matmul(
                            psh[:, m: m + 1],
                            fc1ws[k][:, m * P: (m + 1) * P],
                            fcin[:, k: k + 1],
                            start=(k == 0), stop=(k == 3),
                        )
                nc.vector.tensor_add(hsb, psh, fc1bs)
                nc.vector.tensor_copy(hb16, hsb)

                # ---------- fc2 ----------
                offs = [(0, 512), (512, 512), (1024, 512), (1536, 512), (2048, 256)]
                for (off, nsz) in offs:
                    psw = psr.tile([1, 512], f32, name=f"psw{off}", tag="psr")
                    for k in range(4):
                        nc.tensor.matmul(
                            psw[:, :nsz],
                            hb16[:, k: k + 1],
                            fc2ws[k][:, off: off + nsz],
                            start=(k == 0), stop=(k == 3),
                        )
                    nc.vector.tensor_add(
                        wrow[:, off: off + nsz], psw[:, :nsz], fc2bs[:, off: off + nsz]
                    )
                nc.scalar.activation(out=wrow, in_=wrow, func=Silu)

                # scatter wk [1, 2304] -> [128, 18] via DRAM bounce
                nc.sync.dma_start(out=wkd[:, :, :], in_=wrow)
                for bl in range(2):
                    nc.sync.dma_start(out=wks[:, bl * 9: (bl + 1) * 9], in_=wkd[bl])

                # diagonal dynamic-weight tiles
                for j in range(18):
                    nc.vector.tensor_scalar_mul(diag[:, j, :], ident, wks[:, j: j + 1])

                if DBG:
                    for o in range(2):
                        nc.sync.dma_start(out=dbg_y1[o], in_=y1[o].rearrange("p a b -> p (a b)"))
                    nc.sync.dma_start(out=dbg_fcin[:, :], in_=fcin)
                    nc.sync.dma_start(out=dbg_wks[:, :], in_=wks)

                # ---------- depthwise + alpha + fusion ----------
                taps3 = _taps(1)
                for c in range(NCHUNK):
                    for o in range(2):
                        ps = psb.tile([P, NFREE], f32, name=f"psdw_{o}_{c}", tag="psb")
                        for (t, dy, dx) in taps3:
                            nc.tensor.matmul(
                                ps,
                                diag[:, o * 9 + t, :],
                                xf[o][:, r0(c) + dy: r0(c) + dy + CROWS, 4 + dx: 60 + dx],
                                start=(t == 0), stop=(t == 8),
                            )
                        nc.scalar.copy(fm[o][:, CROWS * c: CROWS * c + CROWS, :], ps)

                    # alpha pre-activation: aw . [f_mod; 0.5*(x1+x2)] + ab
                    pa = psb.tile([1, NFREE], f32, name=f"pa{c}", tag="psb")
                    for o in range(2):
                        nc.tensor.matmul(
                            pa, awms[o], fm[o][:, CROWS * c: CROWS * c + CROWS, :],
                            start=(o == 0), stop=False,
                        )
                    for o in range(2):
                        nc.tensor.matmul(
                            pa, awps[o], s[o][:, CROWS * c: CROWS * c + CROWS, :],
                            start=False, stop=False,
                        )
                    nc.tensor.matmul(pa, abs_, ones392f, start=False, stop=True)
                    arow = mp.tile([1, NFREE], f32, name=f"arow{c}", tag="arow", bufs=2)
                    nc.scalar.copy(arow, pa)
                    pb = psb.tile([P, NFREE], f32, name=f"pb{c}", tag="psb")
                    nc.tensor.matmul(pb, ones, arow, start=True, stop=True)
                    sig = mp.tile([P, CROWS, W], f32, name=f"sig{c}", tag="sig", bufs=2)
                    nc.scalar.activation(out=sig, in_=pb, func=Sigmoid)
                    # sig := alpha = 0.3 + 0.4*sigmoid(z)
                    nc.vector.tensor_scalar(sig, sig, 0.4, 0.3, op0=mult, op1=add)

                    # out = 0.5*s + alpha*(f_mod - 0.5*s), written into s
                    for o in range(2):
                        fmc = fm[o][:, CROWS * c: CROWS * c + CROWS, :]
                        sc = s[o][:, CROWS * c: CROWS * c + CROWS, :]
                        u = mp.tile([P, CROWS, W], f32, name=f"u{c}{o}", tag="u", bufs=3)
                        nc.vector.scalar_tensor_tensor(u, sc, -0.5, fmc, op0=mult, op1=add)
                        nc.vector.tensor_mul(u, u, sig)
                        nc.vector.scalar_tensor_tensor(sc, sc, 0.5, u, op0=mult, op1=add)

                if DBG:
                    for o in range(2):
                        nc.sync.dma_start(out=dbg_fm[o], in_=fm[o].rearrange("p a b -> p (a b)"))
                for o in range(2):
                    nc.sync.dma_start(out=y[o], in_=s[o])

    nc.compile()
    return nc


def _pad_blocks(x, dtype):
    """[C, H, W] fp32 -> [C//128, 128, 64, 64] with zero ring of 4."""
    nb = x.shape[0] // P
    out = np.zeros((nb, P, HP, HP), dtype=dtype)
    out[:, :, 4:60, 4:60] = x.reshape(nb, P, H, W)
    return out


def _prep_host(w1, b1, w2, b2, gw, gb, fc1_w, fc1_b, fc2_w, fc2_b, aw, ab):
    d = {}
    w1tt = np.ascontiguousarray(w1.transpose(1, 2, 3, 0)).reshape(6, P, 9 * HID)
    w2tt = np.ascontiguousarray(w2.transpose(1, 2, 3, 0)).reshape(2, P, 9 * HID)
    d["w1t"] = w1tt.astype(BF16)
    d["w2t"] = w2tt.astype(BF16)
    d["gwt"] = np.ascontiguousarray(gw[:, :, 0, 0].T / 3136.0).reshape(2, P, C).astype(np.float32)
    fc1t = fc1_w.T.copy()          # [2C(k), 512(m)]
    fc1t[C:, :] /= 3136.0          # fold 1/HW for local_pooled half
    d["fc1wt"] = np.ascontiguousarray(fc1t).reshape(4, P, 512).astype(np.float32)
    d["fc2wt"] = np.ascontiguousarray(fc2_w.T).reshape(4, P, C * 9).astype(BF16)
    d["b1row"] = b1.reshape(2, 1, P).astype(BF16)
    d["b2row"] = b2.reshape(2, 1, P).astype(BF16)
    d["gbrow"] = gb.reshape(2, 1, P).astype(np.float32)
    d["fc1bc"] = np.ascontiguousarray(fc1_b.reshape(4, P).T).astype(np.float32)
    d["fc2br"] = fc2_b.reshape(1, C * 9).astype(BF16)
    d["awmc"] = aw[0, :C, 0, 0].reshape(2, P, 1).astype(BF16)
    d["awpc"] = (0.5 * aw[0, C:, 0, 0]).reshape(2, P, 1).astype(np.float32)
    d["abc"] = ab.reshape(1, 1).astype(np.float32)
    d["identc"] = np.eye(P, dtype=np.float32).astype(BF16)
    d["onesc"] = np.ones((1, P), dtype=np.float32)
    d["o392b"] = np.ones((1, NFREE), dtype=np.float32).astype(BF16)
    d["o392f"] = np.ones((1, NFREE), dtype=np.float32)
    return d


def kernel(f_tm2, f_tm1, f_t, w1, b1, w2, b2, gw, gb,
           fc1_w, fc1_b, fc2_w, fc2_b, aw, ab):
    import time

    args = [np.asarray(a, dtype=np.float32) for a in
            (f_tm2, f_tm1, f_t, w1, b1, w2, b2, gw, gb, fc1_w, fc1_b, fc2_w, fc2_b, aw, ab)]
    f_tm2, f_tm1, f_t = args[0], args[1], args[2]

    t0 = time.time()
    shared = _prep_host(*args[3:])
    in_maps = []
    for b in range(B):
        m = dict(shared)
        m["xprev"] = np.concatenate(
            [_pad_blocks(f_tm2[b], BF16), _pad_blocks(f_tm1[b], BF16)], axis=0)
        m["xt"] = _pad_blocks(f_t[b], BF16)
        m["x1r"] = f_tm1[b].reshape(2, P, H, W).astype(np.float32)
        m["x2r"] = f_tm2[b].reshape(2, P, H, W).astype(np.float32)
        in_maps.append(m)
    t1 = time.time()

    nc = build_nc()
    t2 = time.time()
    res = run_bass_kernel_spmd(nc, in_maps, list(range(B)))
    t3 = time.time()

    out = np.stack([res.results[b]["y"].reshape(C, H, W) for b in range(B)]).astype(np.float32)
    LAST_INFO.update(dict(prep_s=t1 - t0, build_s=t2 - t1, run_s=t3 - t2,
                          exec_time_ns=res.exec_time_ns))
    import os as _os
    if _os.environ.get("BASSDBG"):
        LAST_INFO["results"] = res.results
    return out

